# revision 16
# baseline (speedup 1.0000x reference)
"""GroupedQueryAttention kernel for 8 Trainium2 NeuronCores.

Sharding: tensor-parallel over KV groups. Core c owns KV group c
(4 query heads x 64 dim): column shards of w_q/w_k/w_v, row shard of
w_o. x is replicated (passed pre-transposed so the contraction dim
lands on SBUF partitions with zero on-device transposes). Each core
computes a partial output x @ .. @ w_o_shard; host sums the partials.

Per-core math (S=2048 seq, D=2048 model, 4 heads of 64):
  Q^T = wq_c^T x^T          [256, S]   (wq pre-scaled by 1/sqrt(64))
  K^T = wk_c^T x^T          [64, S]
  V^T = wv_c^T x^T -> PE-transpose -> V1 = [V | ones] [S, 65]
  per head h, per seq chunk:
    S^T = K_h Q_h^T         [sk, sq]  (scores, transposed)
    E   = exp(S^T - 8)      (ScalarE, reads PSUM directly)
    O1^T = V1^T E           [65, sq]  -- row 64 = softmax denominator
    O^T = O1^T[0:64] * recip(O1^T[64])  (denominator broadcast via
                                         1-row matmul, DVE multiply)
  Y_partial = O^T^T @ wo_c  (accumulated over 2 k-tiles of 128)

All big matmuls run in float32r (full PE rate at free-dim >= 256,
~1e-4 relative error vs fp32).
"""

import numpy as np

# ---- problem constants (hardcoded per harness contract) ----
S = 2048          # sequence length
D = 2048          # d_model
N_CORES = 8
HD = 64           # head dim
HPG = 4           # heads per KV group (= per core)
QDIM = HPG * HD   # 256, per-core q width
SCALE = 1.0 / 8.0  # 1/sqrt(HD), exact power of two

_compiled = {}


def build_gqa(s=S, d=D, sqc=512, ktg=3, debug=False, debug_taps=False):
    """Build the per-core bass program (SPMD: same program, per-core data)."""
    import concourse.tile as tile
    from concourse import bacc, mybir
    from concourse.masks import make_identity
    from contextlib import ExitStack

    f32 = mybir.dt.float32
    f32r = mybir.dt.float32r
    EXP = mybir.ActivationFunctionType.Exp

    T = s // 128          # seq tiles (sk tiles)
    KO = d // 128         # contraction tiles for projections
    QT = QDIM // 128      # q partition tiles (2)
    NCH = s // sqc        # seq chunks
    och = min(512, d)     # output column chunk width
    NOCH = d // och       # output column chunks

    nc = bacc.Bacc(None, target_bir_lowering=False, debug=debug)
    xT = nc.declare_dram_parameter("xT", [d, s], f32, isOutput=False)
    wq = nc.declare_dram_parameter("wq", [d, QDIM], f32, isOutput=False)
    wk = nc.declare_dram_parameter("wk", [d, HD], f32, isOutput=False)
    wv = nc.declare_dram_parameter("wv", [d, HD], f32, isOutput=False)
    wo = nc.declare_dram_parameter("wo", [QDIM, d], f32, isOutput=False)
    out = nc.declare_dram_parameter("out", [s, d], f32, isOutput=True)
    if debug_taps:
        dbg_qT = nc.declare_dram_parameter("dbg_qT", [64, HPG, s], f32, isOutput=True)
        dbg_kT = nc.declare_dram_parameter("dbg_kT", [64, s], f32, isOutput=True)
        dbg_v1 = nc.declare_dram_parameter("dbg_v1", [128, T, HD + 1], f32, isOutput=True)
        dbg_oT = nc.declare_dram_parameter("dbg_oT", [128, QT, s], f32, isOutput=True)

    with tile.TileContext(nc) as tc, ExitStack() as ctx:
        const = ctx.enter_context(tc.tile_pool(name="const", bufs=1))
        persist = ctx.enter_context(tc.tile_pool(name="persist", bufs=1))

        ident = const.tile([128, 128], f32)
        make_identity(nc, ident)
        ones_stage = const.tile([128, max(T, HD)], f32)
        nc.vector.memset(ones_stage, 1.0)
        ones_row = const.tile([1, HD], f32r)
        nc.sync.dma_start(out=ones_row, in_=ones_stage[0:1, 0:HD].bitcast(f32r))
        bias_exp = const.tile([128, 1], f32)
        nc.vector.memset(bias_exp, -8.0)

        qT_sb = persist.tile([128, QT, s], f32r)
        kT_sb = persist.tile([128, s], f32r)
        v1_sb = persist.tile([128, T, HD + 1], f32r)

        # ---------------- phase 1: projections ----------------
        with (
            tc.tile_pool(name="p1sb", bufs=1) as p1sb,
            tc.tile_pool(name="p1ev", bufs=3) as p1ev,
            tc.tile_pool(name="p1ps", bufs=4, space="PSUM") as p1ps,
            tc.tile_pool(name="vtps", bufs=2, space="PSUM") as vtps,
        ):
            xT_sb = p1sb.tile([128, KO, s], f32r)
            wq_sb = p1sb.tile([128, KO, QDIM], f32r)
            wk_sb = p1sb.tile([128, KO, HD], f32r)
            wv_sb = p1sb.tile([128, KO, HD], f32r)
            nc.sync.dma_start(
                out=wk_sb, in_=wk[:].rearrange("(ko p) m -> p ko m", p=128).bitcast(f32r))
            nc.sync.dma_start(
                out=wv_sb, in_=wv[:].rearrange("(ko p) m -> p ko m", p=128).bitcast(f32r))
            nc.sync.dma_start(
                out=wq_sb, in_=wq[:].rearrange("(ko p) m -> p ko m", p=128).bitcast(f32r))
            for ko in range(KO):
                nc.sync.dma_start(
                    out=xT_sb[:, ko, :],
                    in_=xT[ko * 128:(ko + 1) * 128, :].bitcast(f32r))

            vT_tmp = p1sb.tile([64, s], f32)
            for ch in range(NCH):
                cs = slice(ch * sqc, (ch + 1) * sqc)
                pk = p1ps.tile([128, sqc], f32, name="pk", tag="pp")
                for ko in range(KO):
                    nc.tensor.matmul(pk[0:64, :], wk_sb[:, ko, :], xT_sb[:, ko, cs],
                                     start=(ko == 0), stop=(ko == KO - 1))
                nc.vector.tensor_copy(out=kT_sb[0:64, cs], in_=pk[0:64, :])
                nc.sync.dma_start(out=kT_sb[64:128, cs], in_=kT_sb[0:64, cs])
                pv = p1ps.tile([128, sqc], f32, name="pv", tag="pp")
                for ko in range(KO):
                    nc.tensor.matmul(pv[0:64, :], wv_sb[:, ko, :], xT_sb[:, ko, cs],
                                     start=(ko == 0), stop=(ko == KO - 1))
                nc.vector.tensor_copy(out=vT_tmp[:, cs], in_=pv[0:64, :])

            # V1 = [V | ones]: PE-transpose V^T tiles into natural layout
            nc.sync.dma_start(out=v1_sb[:, :, HD:HD + 1],
                              in_=ones_stage[:, 0:T, None].bitcast(f32r))
            for t in range(T):
                pt = vtps.tile([128, HD], f32, name="pt")
                nc.tensor.transpose(
                    pt, vT_tmp[:, t * 128:(t + 1) * 128], ident[:64, :64])
                nc.vector.tensor_copy(out=v1_sb[:, t, 0:HD], in_=pt)

            for qt in range(QT):
                for ch in range(NCH):
                    cs = slice(ch * sqc, (ch + 1) * sqc)
                    pq = p1ps.tile([128, sqc], f32, name="pq", tag="pp")
                    for ko in range(KO):
                        nc.tensor.matmul(
                            pq, wq_sb[:, ko, qt * 128:(qt + 1) * 128],
                            xT_sb[:, ko, cs],
                            start=(ko == 0), stop=(ko == KO - 1))
                    nc.vector.tensor_copy(out=qT_sb[:, qt, cs], in_=pq)

        # ---------------- phase 2+3: attention + output proj ----------------
        p2sb = ctx.enter_context(tc.tile_pool(name="p2sb", bufs=3))
        p2o = ctx.enter_context(tc.tile_pool(name="p2o", bufs=1))
        oT_sb = p2o.tile([128, QT, s], f32r)
        p2w = ctx.enter_context(tc.tile_pool(name="p2w", bufs=1))
        p2ev = ctx.enter_context(tc.tile_pool(name="p2ev", bufs=3))
        scps = ctx.enter_context(tc.tile_pool(name="scps", bufs=2, space="PSUM"))
        otps = ctx.enter_context(tc.tile_pool(name="otps", bufs=1, space="PSUM"))
        mips = ctx.enter_context(tc.tile_pool(name="mips", bufs=1, space="PSUM"))

        wo_sb = p2w.tile([128, QT, d], f32r)
        nc.sync.dma_start(
            out=wo_sb, in_=wo[:].rearrange("(ko p) m -> p ko m", p=128).bitcast(f32r))

        # kt group sizes, e.g. T=16, ktg=3 -> [3,3,3,3,3,1]
        groups = []
        i = 0
        while i < T:
            groups.append(min(ktg, T - i))
            i += ktg

        for ch in range(NCH):
            cs = slice(ch * sqc, (ch + 1) * sqc)
            for h in range(HPG):
                hp = 64 * (h % 2)
                qh = qT_sb[hp:hp + 64, h // 2, cs]          # [64, sqc]
                ot_ps = otps.tile([128, sqc], f32, name="otp")
                kt = 0
                for gsz in groups:
                    sc_ps = scps.tile([128, ktg, sqc], f32, name="scp")
                    for j in range(gsz):
                        nc.tensor.matmul(
                            sc_ps[:, j, :],
                            kT_sb[hp:hp + 64, (kt + j) * 128:(kt + j + 1) * 128],
                            qh, start=True, stop=True)
                    e_sb = p2sb.tile([128, ktg, sqc], f32r, name="e_sb")
                    nc.scalar.activation(
                        out=e_sb[:, :gsz, :], in_=sc_ps[:, :gsz, :],
                        func=EXP, bias=bias_exp, scale=1.0)
                    for j in range(gsz):
                        nc.tensor.matmul(
                            ot_ps[0:HD + 1, :], v1_sb[:, kt + j, :],
                            e_sb[:, j, :],
                            start=(kt + j == 0), stop=(kt + j == T - 1))
                    kt += gsz
                # epilogue: normalize by the ones-column denominator
                recip = p2ev.tile([1, sqc], f32r, name="recip")
                with nc.allow_low_precision(reason="f32r rounding of softmax recip, ~1e-4 rel"):
                    nc.vector.reciprocal(out=recip, in_=ot_ps[HD:HD + 1, :])
                bc_ps = mips.tile([128, sqc], f32, name="bcp", tag="mip")
                nc.tensor.matmul(bc_ps[0:HD, :], ones_row, recip, start=True, stop=True)
                onorm = p2ev.tile([64, sqc], f32, name="onorm")
                nc.vector.tensor_copy(out=onorm, in_=ot_ps[0:HD, :])
                nc.vector.tensor_mul(out=onorm, in0=onorm, in1=bc_ps[0:HD, :])
                nc.sync.dma_start(
                    out=oT_sb[hp:hp + 64, h // 2, cs],
                    in_=onorm[:].bitcast(f32r))
            # output projection for the seq tiles of this chunk
            for tt in range(sqc // 128):
                t = ch * (sqc // 128) + tt
                for nch2 in range(NOCH):
                    ns = slice(nch2 * och, (nch2 + 1) * och)
                    py = mips.tile([128, och], f32, name="pyp", tag="mip")
                    for qt in range(QT):
                        nc.tensor.matmul(
                            py, oT_sb[:, qt, t * 128:(t + 1) * 128],
                            wo_sb[:, qt, ns],
                            start=(qt == 0), stop=(qt == QT - 1))
                    y_sb = p2ev.tile([128, och], f32, name="y_sb")
                    nc.vector.tensor_copy(out=y_sb, in_=py)
                    nc.sync.dma_start(
                        out=out[:].rearrange("(t p) n -> p t n", p=128)[:, t, ns],
                        in_=y_sb)

        if debug_taps:
            nc.sync.dma_start(out=dbg_qT[:], in_=qT_sb[:].bitcast(f32))
            nc.sync.dma_start(out=dbg_kT[:], in_=kT_sb[:].bitcast(f32))
            nc.sync.dma_start(out=dbg_v1[:], in_=v1_sb[:].bitcast(f32))
            nc.sync.dma_start(out=dbg_oT[:], in_=oT_sb[:].bitcast(f32))

    nc.compile()
    return nc


def _get_nc():
    if "nc" not in _compiled:
        _compiled["nc"] = build_gqa()
    return _compiled["nc"]


def _shard_inputs(x, w_q, w_k, w_v, w_o):
    x = np.asarray(x, dtype=np.float32)
    w_q = np.asarray(w_q, dtype=np.float32)
    w_k = np.asarray(w_k, dtype=np.float32)
    w_v = np.asarray(w_v, dtype=np.float32)
    w_o = np.asarray(w_o, dtype=np.float32)
    xT = np.ascontiguousarray(x.reshape(S, D).T)
    in_maps = []
    for c in range(N_CORES):
        in_maps.append({
            "xT": xT,
            "wq": np.ascontiguousarray(w_q[:, c * QDIM:(c + 1) * QDIM]) * np.float32(SCALE),
            "wk": np.ascontiguousarray(w_k[:, c * HD:(c + 1) * HD]),
            "wv": np.ascontiguousarray(w_v[:, c * HD:(c + 1) * HD]),
            "wo": np.ascontiguousarray(w_o[c * QDIM:(c + 1) * QDIM, :]),
        })
    return in_maps


def kernel(x, w_q, w_k, w_v, w_o):
    from concourse.bass_utils import run_bass_kernel_spmd

    nc = _get_nc()
    in_maps = _shard_inputs(x, w_q, w_k, w_v, w_o)
    res = run_bass_kernel_spmd(nc, in_maps, list(range(N_CORES)))
    acc = np.zeros((S, D), dtype=np.float64)
    for r in res.results:
        acc += r["out"].astype(np.float64)
    return acc.astype(np.float32).reshape(1, S, D)
